# revision 1
# baseline (speedup 1.0000x reference)
"""Trainium2 Bass kernel for sliding-window GQA attention block.

Problem: B=1, S=2048, HID=2048, NH=16 q-heads, NKV=4 kv-heads, HD=128,
WINDOW=512, causal; rotary embedding on q/k; projections wq/wk/wv/wo.

Sharding (8 cores): tensor-parallel over the 4 KV-head groups (4 q-heads
per group) x sequence-parallel over 2 halves of 1024 queries. Each core
computes its group's q/k/v projections for its sequence span (+512-key
halo), banded sliding-window attention, and a partial output projection.
Host sums the 4 group-partials per half.

All matmuls run as float32r (probed: same numerics as the PE fp32 path,
4x faster). RoPE is done with DVE muls against stacked cos/sin tiles plus
a constant [[I],[+-I]] combine matmul on the PE (cross-partition shuffles
are illegal on DVE). Masking is geometric: two 0/1 boundary tiles applied
to exp(scores), plus zeroed rows in the all-ones denominator matmul for
the r=0 halo padding.
"""
import sys
import os

sys.path.insert(0, "/opt/trn_rl_repo")

import numpy as np

import concourse.bass as bass
import concourse.mybir as mybir
from concourse import bacc
import concourse.tile as tile
from concourse.bass_utils import run_bass_kernel_spmd

F32R = mybir.dt.float32r
F32 = mybir.dt.float32

S, HID, NH, NKV, HD, WINDOW = 2048, 2048, 16, 4, 128, 512
NCORES = 8
SQ = 1024          # queries per core
SK = 1536          # keys per core (incl. 512 halo)
HT = HID // 128    # 16 hid tiles
NHC = NH // NKV    # 4 q-heads per core
TJ = SK // 128     # 12 key tiles
EXP = mybir.ActivationFunctionType.Exp


def _win(tj):
    """Query window [w0, w1) of key tile tj in core-local coordinates."""
    return max(0, 128 * tj - 512), min(SQ, 128 * tj + 128)


_P_OFF = []
_off = 0
for _tj in range(TJ):
    _w0, _w1 = _win(_tj)
    _P_OFF.append(_off)
    _off += _w1 - _w0
P_TOTAL = _off  # 5120


def build_nc():
    nc = bacc.Bacc("TRN2", target_bir_lowering=False, debug=False)

    xt_d = nc.dram_tensor("xt", [HID, SK], F32R, kind="ExternalInput").ap()
    wqt_d = nc.dram_tensor("wqt", [HID, 512], F32R, kind="ExternalInput").ap()
    wkt_d = nc.dram_tensor("wkt", [HID, 128], F32R, kind="ExternalInput").ap()
    wvt_d = nc.dram_tensor("wvt", [HID, 128], F32R, kind="ExternalInput").ap()
    wot_d = nc.dram_tensor("wot", [512, HID], F32R, kind="ExternalInput").ap()
    csa_d = nc.dram_tensor("csa", [128, SK], F32, kind="ExternalInput").ap()
    csb_d = nc.dram_tensor("csb", [128, SK], F32, kind="ExternalInput").ap()
    comb_d = nc.dram_tensor("comb", [128, 256], F32R, kind="ExternalInput").ap()
    ident_d = nc.dram_tensor("ident", [128, 128], F32R, kind="ExternalInput").ap()
    onesm_d = nc.dram_tensor("onesm", [128, SK], F32R, kind="ExternalInput").ap()
    bnd_d = nc.dram_tensor("bnd", [128, 256], F32, kind="ExternalInput").ap()
    out_d = nc.dram_tensor("out", [SQ, HID], F32, kind="ExternalOutput").ap()

    with tile.TileContext(nc) as tc:
        with tc.tile_pool(name="persist", bufs=1) as pp:
            wkt_sb = pp.tile([128, HT * 128], F32R)    # 8KB
            wvt_sb = pp.tile([128, HT * 128], F32R)    # 8KB
            csa_sb = pp.tile([128, SK], F32)           # 6KB
            csb_sb = pp.tile([128, SK], F32)           # 6KB
            comb_sb = pp.tile([128, 256], F32R)
            ident_sb = pp.tile([128, 128], F32R)
            onesm_sb = pp.tile([128, SK], F32R)        # 6KB
            bnd_sb = pp.tile([128, 256], F32)
            kt_rot = pp.tile([128, SK], F32R)          # 6KB
            vs_sb = pp.tile([128, SK], F32R)           # 6KB
            qt_rot = pp.tile([128, NHC * SQ], F32R)    # 16KB
            attnT = pp.tile([128, NHC * SQ], F32R)     # 16KB
            vt_sb = pp.tile([128, SK], F32R)           # 6KB v staging (d-major)
            m1_sb = pp.tile([128, 512], F32R)
            m2_sb = pp.tile([128, 512], F32R)
            m1b_sb = pp.tile([128, 512], F32R)
            m2b_sb = pp.tile([128, 512], F32R)
            recip_sb = pp.tile([128, 512], F32)

            # ---- load persistent inputs ----
            for _a, _b in [(0, 2), (2, 9), (9, 16)]:
                _n = _b - _a
                _sl = slice(128 * _a, 128 * _b)
                nc.sync.dma_start(
                    wkt_sb[:, _sl].rearrange("p (t f) -> p t f", t=_n),
                    wkt_d[128 * _a:128 * _b, :].rearrange(
                        "(t p) f -> p t f", p=128))
                nc.sync.dma_start(
                    wvt_sb[:, _sl].rearrange("p (t f) -> p t f", t=_n),
                    wvt_d[128 * _a:128 * _b, :].rearrange(
                        "(t p) f -> p t f", p=128))
            nc.sync.dma_start(comb_sb[:], comb_d)
            nc.sync.dma_start(ident_sb[:], ident_d)

            # ================= phase 2: k/v projections =================
            with tc.tile_pool(name="wqph", bufs=1) as wqph:
             wqt_sb = wqph.tile([128, HT * 512], F32R)   # 32KB, dies after ph3
             with tc.tile_pool(name="xkv", bufs=8) as xkvp, \
                  tc.tile_pool(name="kvps", bufs=1, space="PSUM") as kvps, \
                  tc.tile_pool(name="rotps", bufs=1, space="PSUM") as rotps:
                 k_ps = kvps.tile([128, SK], F32, tag="k")
                 v_ps = kvps.tile([128, SK], F32, tag="v")
                 for ht in range(HT):
                     xt_t = xkvp.tile([128, SK], F32R, tag="x")
                     if ht == 0:
                         nc.sync.dma_start(xt_t[:, 0:512], xt_d[0:128, 0:512])
                         nc.sync.dma_start(xt_t[:, 512:SK], xt_d[0:128, 512:SK])
                     else:
                         nc.sync.dma_start(xt_t[:],
                                           xt_d[128 * ht:128 * (ht + 1), :])
                     for sc in range(3):
                         sl = slice(512 * sc, 512 * (sc + 1))
                         nc.tensor.matmul(k_ps[:, sl],
                                          wkt_sb[:, 128 * ht:128 * (ht + 1)],
                                          xt_t[:, sl],
                                          start=(ht == 0), stop=(ht == HT - 1))
                         nc.tensor.matmul(v_ps[:, sl],
                                          wvt_sb[:, 128 * ht:128 * (ht + 1)],
                                          xt_t[:, sl],
                                          start=(ht == 0), stop=(ht == HT - 1))
                 nc.sync.dma_start(csa_sb[:], csa_d)
                 nc.sync.dma_start(csb_sb[:], csb_d)
                 for _t in range(HT):
                     nc.sync.dma_start(
                         wqt_sb[:, 512 * _t:512 * (_t + 1)],
                         wqt_d[128 * _t:128 * (_t + 1), :])
                 nc.sync.dma_start(onesm_sb[:], onesm_d)
                 nc.sync.dma_start(bnd_sb[:], bnd_d)
                 # rope(k) -> kt_rot (combine matmuls reuse the k psum banks)
                 for sc in range(3):
                     sl = slice(512 * sc, 512 * (sc + 1))
                     ma = m1_sb if sc % 2 == 0 else m1b_sb
                     mb = m2_sb if sc % 2 == 0 else m2b_sb
                     nc.vector.tensor_mul(ma[:], k_ps[:, sl], csa_sb[:, sl])
                     nc.vector.tensor_mul(mb[:], k_ps[:, sl], csb_sb[:, sl])
                     nc.tensor.matmul(k_ps[:, sl], comb_sb[:, 0:128], ma[:],
                                      start=True, stop=False)
                     nc.tensor.matmul(k_ps[:, sl], comb_sb[:, 128:256], mb[:],
                                      start=False, stop=True)
                     nc.scalar.copy(kt_rot[:, sl], k_ps[:, sl])
                 # v: psum -> sbuf (d-major), transpose to s-major
                 for sc in range(3):
                     sl = slice(512 * sc, 512 * (sc + 1))
                     nc.scalar.copy(vt_sb[:, sl], v_ps[:, sl])
                 for tj in range(TJ):
                     sl = slice(128 * tj, 128 * (tj + 1))
                     t_ps = rotps.tile([128, 128], F32R, tag="tr")
                     nc.tensor.transpose(t_ps[:], vt_sb[:, sl], ident_sb[:])
                     nc.vector.tensor_copy(vs_sb[:, sl], t_ps[:])

             # ================ phase 3: q projection + rope =================
             with tc.tile_pool(name="xq", bufs=8) as xqp, \
                  tc.tile_pool(name="qps", bufs=1, space="PSUM") as qps:
                 q_ps = [qps.tile([128, 512], F32, tag=f"q{i}", name=f"q_ps{i}")
                         for i in range(8)]
                 for ht in range(HT):
                     xq_t = xqp.tile([128, 1024], F32R, tag="xq")
                     nc.sync.dma_start(
                         xq_t[:], xt_d[128 * ht:128 * (ht + 1), 512:SK])
                     for ot in range(NHC):
                         for sc in range(2):
                             nc.tensor.matmul(
                                 q_ps[2 * ot + sc][:],
                                 wqt_sb[:, 512 * ht + 128 * ot:
                                        512 * ht + 128 * (ot + 1)],
                                 xq_t[:, 512 * sc:512 * (sc + 1)],
                                 start=(ht == 0), stop=(ht == HT - 1))
                 for ot in range(NHC):
                     for sc in range(2):
                         i = 2 * ot + sc
                         cs_sl = slice(512 + 512 * sc, 1024 + 512 * sc)
                         ma = m1_sb if i % 2 == 0 else m1b_sb
                         mb = m2_sb if i % 2 == 0 else m2b_sb
                         nc.vector.tensor_mul(ma[:], q_ps[i][:], csa_sb[:, cs_sl])
                         nc.vector.tensor_mul(mb[:], q_ps[i][:], csb_sb[:, cs_sl])
                         nc.tensor.matmul(q_ps[i][:], comb_sb[:, 0:128], ma[:],
                                          start=True, stop=False)
                         nc.tensor.matmul(q_ps[i][:], comb_sb[:, 128:256], mb[:],
                                          start=False, stop=True)
                         nc.scalar.copy(
                             qt_rot[:, SQ * ot + 512 * sc:SQ * ot + 512 * (sc + 1)],
                             q_ps[i][:])

            # ============ phases 4+5 share the preloaded wot ============
            wotp_cm = tc.tile_pool(name="wotp", bufs=1)
            wotp = wotp_cm.__enter__()
            wot_sb = wotp.tile([128, 16 * 512], F32R)  # 32KB, all of wot
            for _m in range(NHC):
                for _oc in range(4):
                    nc.sync.dma_start(
                        wot_sb[:, 2048 * _m + 512 * _oc:
                               2048 * _m + 512 * (_oc + 1)],
                        wot_d[128 * _m:128 * (_m + 1),
                              512 * _oc:512 * (_oc + 1)])
            # ================= phase 4: banded attention =================
            with tc.tile_pool(name="pbl", bufs=1) as pbl, \
                 tc.tile_pool(name="sps", bufs=2, space="PSUM") as sps, \
                 tc.tile_pool(name="ops", bufs=2, space="PSUM") as ops:
                pblocks = [pbl.tile([128, P_TOTAL], F32R, tag=f"pb{i}",
                                    name=f"pblock{i}") for i in range(2)]
                def scores_part(h):
                    pblock = pblocks[h % 2]
                    # scores + exp + masks; tj=4 and tj=8 first: they are the
                    # start=True full-coverage tiles gating the chunk psums
                    for tj in (4, 8, 0, 1, 2, 3, 5, 6, 7, 9, 10, 11):
                        w0, w1 = _win(tj)
                        W = w1 - w0
                        s_ps = sps.tile([128, 768], F32, tag="s")
                        ktile = kt_rot[:, 128 * tj:128 * (tj + 1)]
                        # pad narrow windows: fp32r needs N>=256 per matmul
                        # piece, and each piece must sit in one psum bank.
                        if W < 256:
                            Wp = 256
                        elif W <= 512:
                            Wp = W
                        else:
                            Wp = 768
                        w0p = max(0, min(w0, SQ - Wp))
                        d0 = w0 - w0p
                        qv = qt_rot[:, SQ * h + w0p:SQ * h + w0p + Wp]
                        if Wp <= 512:
                            nc.tensor.matmul(s_ps[:, 0:Wp], ktile, qv,
                                             start=True, stop=True)
                        else:
                            nc.tensor.matmul(s_ps[:, 0:512], ktile,
                                             qv[:, 0:512],
                                             start=True, stop=True)
                            nc.tensor.matmul(s_ps[:, 512:768], ktile,
                                             qv[:, 512:768],
                                             start=True, stop=True)
                        pt = pblock[:, _P_OFF[tj]:_P_OFF[tj] + W]
                        nc.scalar.activation(pt, s_ps[:, d0:d0 + W], EXP)
                        if tj >= 4:
                            nc.gpsimd.tensor_mul(pt[:, 0:128], pt[:, 0:128],
                                                 bnd_sb[:, 0:128])
                        if tj <= 7:
                            nc.gpsimd.tensor_mul(pt[:, W - 128:W], pt[:, W - 128:W],
                                                 bnd_sb[:, 128:256])
                def chunks_part(h):
                    pblock = pblocks[h % 2]
                    # denominator + PV accumulation per 512-query chunk
                    for c in range(2):
                        o_ps = ops.tile([128, 512], F32, tag="o")
                        den_ps = ops.tile([128, 512], F32, tag="den")
                        order = [4 * c + 4] + [4 * c + k for k in (0, 1, 2, 3, 5, 6, 7)]
                        for idx, tj in enumerate(order):
                            w0, w1 = _win(tj)
                            W = w1 - w0
                            lo = max(0, 512 * c - w0)
                            hi = min(W, 512 * c + 512 - w0)
                            pc = slice(w0 + lo - 512 * c, w0 + hi - 512 * c)
                            prhs = pblock[:, _P_OFF[tj] + lo:_P_OFF[tj] + hi]
                            st, sp = idx == 0, idx == len(order) - 1
                            nc.tensor.matmul(den_ps[:, pc],
                                             onesm_sb[:, 128 * tj:128 * (tj + 1)],
                                             prhs, start=st, stop=sp,
                                             skip_group_check=True)
                            nc.tensor.matmul(o_ps[:, pc],
                                             vs_sb[:, 128 * tj:128 * (tj + 1)],
                                             prhs, start=st, stop=sp,
                                             skip_group_check=True)
                        nc.vector.reciprocal_approx_fast(recip_sb[:], den_ps[:])
                        nc.vector.tensor_mul(
                            attnT[:, SQ * h + 512 * c:SQ * h + 512 * (c + 1)],
                            o_ps[:], recip_sb[:])

                scores_part(0)
                for h in range(1, NHC):
                    scores_part(h)
                    chunks_part(h - 1)
                chunks_part(NHC - 1)

            # ================= phase 5: output projection =================
            with tc.tile_pool(name="ost", bufs=1) as ostp, \
                 tc.tile_pool(name="fps", bufs=1, space="PSUM") as fps:
                for oc in range(4):
                    f_ps = [fps.tile([128, 512], F32, tag=f"f{st}", name=f"f_ps{st}")
                            for st in range(8)]
                    for m in range(NHC):
                        wot_t = wot_sb[:, 2048 * m + 512 * oc:
                                       2048 * m + 512 * (oc + 1)]
                        for st in range(8):
                            nc.tensor.matmul(
                                f_ps[st][:],
                                attnT[:, SQ * m + 128 * st:SQ * m + 128 * (st + 1)],
                                wot_t,
                                start=(m == 0), stop=(m == NHC - 1))
                    for st in range(8):
                        stage = ostp.tile([128, 512], F32, tag="st", bufs=4)
                        if st % 2 == 0:
                            nc.scalar.copy(stage[:], f_ps[st][:])
                        else:
                            nc.vector.tensor_copy(stage[:], f_ps[st][:])
                        nc.sync.dma_start(
                            out_d[128 * st:128 * (st + 1), 512 * oc:512 * (oc + 1)],
                            stage[:])
            wotp_cm.__exit__(None, None, None)

    nc.compile()
    return nc


def host_inputs(x, wq, wk, wv, wo, freqs_cos, freqs_sin):
    """Build the 8 per-core input dicts."""
    xT = np.ascontiguousarray(np.asarray(x, dtype=np.float32)[0].T)  # [hid, s]
    wq = np.asarray(wq, dtype=np.float32)
    wk = np.asarray(wk, dtype=np.float32)
    wv = np.asarray(wv, dtype=np.float32)
    wo = np.asarray(wo, dtype=np.float32)
    cosT = np.asarray(freqs_cos, dtype=np.float32).T  # [64, S]
    sinT = np.asarray(freqs_sin, dtype=np.float32).T

    comb = np.zeros((128, 256), dtype=np.float32)
    for p in range(64):
        comb[p, p] = 1.0        # C1: out[p] = m1[p] - m1[p+64]
        comb[64 + p, p] = -1.0
        comb[p, 128 + 64 + p] = 1.0   # C2: out[64+p] = m2[p] + m2[p+64]
        comb[64 + p, 128 + 64 + p] = 1.0
    ident = np.eye(128, dtype=np.float32)
    y = np.arange(128)[None, :]
    xg = np.arange(128)[:, None]
    bnd = np.concatenate([(y >= xg).astype(np.float32),
                          (y <= xg).astype(np.float32)], axis=1)  # [128, 256]

    in_maps = []
    for core in range(NCORES):
        g, r = core // 2, core % 2
        lo = 1024 * r - 512
        xt = np.zeros((HID, SK), dtype=np.float32)
        if r == 0:
            xt[:, 512:] = xT[:, 0:1024]
        else:
            xt[:, :] = xT[:, 512:2048]
        pos = np.clip(np.arange(lo, lo + SK), 0, S - 1)
        csa = np.concatenate([cosT[:, pos], sinT[:, pos]], axis=0)
        csb = np.concatenate([sinT[:, pos], cosT[:, pos]], axis=0)
        onesm = np.zeros((128, SK), dtype=np.float32)
        for tj in range(TJ):
            real = np.ones(128, dtype=np.float32) if r == 1 else \
                (128 * tj + np.arange(128) >= 512).astype(np.float32)
            onesm[:, 128 * tj:128 * (tj + 1)] = real[:, None]
        in_maps.append({
            "xt": np.ascontiguousarray(xt),
            "wqt": np.ascontiguousarray(wq[512 * g:512 * (g + 1), :].T),
            "wkt": np.ascontiguousarray(wk[128 * g:128 * (g + 1), :].T
                                        / np.sqrt(HD)),
            "wvt": np.ascontiguousarray(wv[128 * g:128 * (g + 1), :].T),
            "wot": np.ascontiguousarray(wo[:, 512 * g:512 * (g + 1)].T),
            "csa": np.ascontiguousarray(csa),
            "csb": np.ascontiguousarray(csb),
            "comb": comb,
            "ident": ident,
            "onesm": onesm,
            "bnd": bnd,
        })
    return in_maps


def reduce_outputs(results):
    out = np.zeros((S, HID), dtype=np.float32)
    for core, res in enumerate(results):
        r = core % 2
        out[1024 * r:1024 * (r + 1), :] += res["out"]
    return out[None]


_NC = None
_IN_MAPS = None


def _numpy_fallback(x, wq, wk, wv, wo, attention_mask, freqs_cos, freqs_sin):
    """Exact (slow) path for non-causal attention_mask inputs."""
    xs = np.asarray(x, np.float32)[0]
    cos = np.asarray(freqs_cos, np.float32)
    sin = np.asarray(freqs_sin, np.float32)

    def rope(t):
        x1, x2 = t[..., :64], t[..., 64:]
        c, s = cos[:, None, :], sin[:, None, :]
        return np.concatenate([x1 * c - x2 * s, x1 * s + x2 * c], axis=-1)

    q = rope((xs @ np.asarray(wq, np.float32).T).reshape(S, NH, HD))
    k = rope((xs @ np.asarray(wk, np.float32).T).reshape(S, NKV, HD))
    v = (xs @ np.asarray(wv, np.float32).T).reshape(S, NKV, HD)
    k = np.repeat(k, NH // NKV, axis=1)
    v = np.repeat(v, NH // NKV, axis=1)
    i = np.arange(S)[:, None]
    j = np.arange(S)[None, :]
    wmask = (i - j) > WINDOW
    out = np.zeros((S, NH, HD), np.float32)
    am = np.asarray(attention_mask, np.float32)[0, 0]
    for h in range(NH):
        sc = (q[:, h] @ k[:, h].T) / np.sqrt(HD) + am
        sc = np.where(wmask, -np.inf, sc)
        sc -= sc.max(axis=1, keepdims=True)
        p = np.exp(sc)
        p /= p.sum(axis=1, keepdims=True)
        out[:, h] = p @ v[:, h]
    return (out.reshape(S, NH * HD) @ np.asarray(wo, np.float32).T)[None]


def _is_standard_causal(attention_mask):
    am = np.asarray(attention_mask)
    if am.shape != (1, 1, S, S):
        return False
    i = np.arange(S)[:, None]
    j = np.arange(S)[None, :]
    expect = np.where(j > i, np.float32(-1e9), np.float32(0.0))
    return np.array_equal(am[0, 0], expect)


def kernel(x, wq, wk, wv, wo, attention_mask, freqs_cos, freqs_sin,
           **extra):
    global _NC, _IN_MAPS
    if not _is_standard_causal(attention_mask):
        return _numpy_fallback(x, wq, wk, wv, wo, attention_mask,
                               freqs_cos, freqs_sin)
    in_maps = host_inputs(x, wq, wk, wv, wo, freqs_cos, freqs_sin)
    _IN_MAPS = in_maps
    if _NC is None:
        _NC = build_nc()
    res = run_bass_kernel_spmd(_NC, in_maps, core_ids=list(range(NCORES)))
    return reduce_outputs(res.results)


if __name__ == "__main__":
    nc = build_nc()
    print("kernel built OK")



# revision 3
# speedup vs baseline: 1.0948x; 1.0948x over previous
"""Trainium2 Bass kernel for sliding-window GQA attention block (bf16).

Problem: B=1, S=2048, HID=2048, NH=16 q-heads, NKV=4 kv-heads, HD=128,
WINDOW=512, causal; rotary embedding on q/k; projections wq/wk/wv/wo.

Sharding (8 cores): tensor-parallel over the 4 KV-head groups (4 q-heads
per group) x sequence-parallel over 2 halves of 1024 queries. Each core
computes its group's q/k/v projections for its sequence span (+512-key
halo), banded sliding-window attention, and a partial output projection.
Host sums the 4 group-partials per half.

v2 (bf16 rewrite of the fp32r baseline):
- All matmul operands bf16 (fp32 PSUM accumulation). Halves HBM traffic
  and removes the fp32r N<256 4x matmul penalty.
- x is DMA'd once (bf16, host-transposed) and stays resident in SBUF;
  both the k/v and q projections stream it from there.
- Weights are pre-laid out on the host in the exact SBUF layout so every
  DMA is a single contiguous transfer (the fp32r baseline lost ~19us to
  strided weight gathers before the first matmul).
- Scores for each (chunk, head) are packed into three PSUM tiles
  (1024/1024/512 wide, pieces bank-aligned) so exp() runs as 3 large
  ACT calls instead of 12 small ones.
- Chunk-outer / head-inner attention ordering: the output projection for
  query-half 0 overlaps attention for query-half 1 in PSUM.
- Output partials are bf16 (host accumulates in fp32).
"""
import sys
import os

sys.path.insert(0, "/opt/trn_rl_repo")

import numpy as np
import ml_dtypes

import concourse.bass as bass
import concourse.mybir as mybir
from concourse import bacc
import concourse.tile as tile
from concourse.bass_utils import run_bass_kernel_spmd

BF = mybir.dt.bfloat16
F32 = mybir.dt.float32

S, HID, NH, NKV, HD, WINDOW = 2048, 2048, 16, 4, 128, 512
NCORES = 8
SQ = 1024          # queries per core
SK = 1536          # keys per core (incl. 512 halo)
HT = HID // 128    # 16 hid tiles
NHC = NH // NKV    # 4 q-heads per core
TJ = SK // 128     # 12 key tiles
EXP = mybir.ActivationFunctionType.Exp
BF_NP = ml_dtypes.bfloat16


def _win(tj):
    """Query window [w0, w1) of key tile tj in core-local coordinates."""
    return max(0, 128 * tj - 512), min(SQ, 128 * tj + 128)


def _piece(tj, c):
    """Clip key-tile tj's query window to chunk c. -> (qlo, n) or None."""
    w0, w1 = _win(tj)
    lo = max(0, 512 * c - w0)
    hi = min(w1 - w0, 512 * c + 512 - w0)
    if hi <= lo:
        return None
    return w0 + lo, hi - lo


# Scores/pblock layout per chunk c (a=4c): three bank-aligned psum tiles.
# T0: [a+3:512 @0 | a+2:384 @512, a+0:128 @896]
# T1: [a+4:512 @0 | a+5:384 @512, a+7:128 @896]
# T2: [a+1:256 @0 | a+6:256 @256]
# pb offsets: T0 -> +0, T1 -> +1024, T2 -> +2048 (2560 per chunk-head).
def _tiles(c):
    a = 4 * c
    return [
        (1024, [(a + 3, 0), (a + 2, 512), (a + 0, 896)]),
        (1024, [(a + 4, 0), (a + 5, 512), (a + 7, 896)]),
        (512, [(a + 1, 0), (a + 6, 256)]),
    ]


# (pb_offset, n) inside the 2560-wide chunk block, by tj (for den/pv).
def _pb_off(c):
    out = {}
    for ti, (_, pieces) in enumerate(_tiles(c)):
        base = (0, 1024, 2048)[ti]
        for tj, off in pieces:
            qlo, n = _piece(tj, c)
            out[tj] = (base + off, qlo, n)
    return out


def build_nc():
    nc = bacc.Bacc("TRN2", target_bir_lowering=False, debug=False)

    xt_d = nc.dram_tensor("xt", [HID, SK], BF, kind="ExternalInput").ap()
    wqt_d = nc.dram_tensor("wqt", [128, HT * 512], BF, kind="ExternalInput").ap()
    wkt_d = nc.dram_tensor("wkt", [128, HT * 128], BF, kind="ExternalInput").ap()
    wvt_d = nc.dram_tensor("wvt", [128, HT * 128], BF, kind="ExternalInput").ap()
    wot_d = nc.dram_tensor("wot", [128, 16 * 512], BF, kind="ExternalInput").ap()
    csa_d = nc.dram_tensor("csa", [128, SK], F32, kind="ExternalInput").ap()
    csb_d = nc.dram_tensor("csb", [128, SK], F32, kind="ExternalInput").ap()
    comb_d = nc.dram_tensor("comb", [128, 256], BF, kind="ExternalInput").ap()
    ident_d = nc.dram_tensor("ident", [128, 128], BF, kind="ExternalInput").ap()
    onesm_d = nc.dram_tensor("onesm", [128, SK], BF, kind="ExternalInput").ap()
    bnd_d = nc.dram_tensor("bnd", [128, 256], BF, kind="ExternalInput").ap()
    out_d = nc.dram_tensor("out", [SQ, HID], BF, kind="ExternalOutput").ap()

    with tile.TileContext(nc) as tc:
        with tc.tile_pool(name="persist", bufs=1) as pp:
            wkt_sb = pp.tile([128, HT * 128], BF)
            wvt_sb = pp.tile([128, HT * 128], BF)
            wqt_sb = pp.tile([128, HT * 512], BF)
            wot_sb = pp.tile([128, 16 * 512], BF)
            xt_sb = pp.tile([128, HT * SK], BF)     # 48KB/part, resident x
            csa_sb = pp.tile([128, SK], F32)
            csb_sb = pp.tile([128, SK], F32)
            comb_sb = pp.tile([128, 256], BF)
            ident_sb = pp.tile([128, 128], BF)
            onesm_sb = pp.tile([128, SK], BF)
            bnd_sb = pp.tile([128, 256], BF)
            kt_rot = pp.tile([128, SK], BF)
            vs_sb = pp.tile([128, SK], BF)
            vt_sb = pp.tile([128, SK], BF)
            qt_rot = pp.tile([128, NHC * SQ], BF)
            attnT = pp.tile([128, NHC * SQ], BF)
            m1_sb = pp.tile([128, 512], BF)
            m2_sb = pp.tile([128, 512], BF)
            m1b_sb = pp.tile([128, 512], BF)
            m2b_sb = pp.tile([128, 512], BF)
            recip_sb = pp.tile([128, 512], F32)

            # ---- priming DMAs (order = consumption order) ----
            nc.sync.dma_start(wkt_sb[:], wkt_d)
            nc.sync.dma_start(xt_sb[:, 0:SK], xt_d[0:128, :])
            nc.sync.dma_start(wvt_sb[:], wvt_d)

            # ================= phase 1: k/v projections =================
            with tc.tile_pool(name="kvps", bufs=1, space="PSUM") as kvps, \
                 tc.tile_pool(name="rotps", bufs=2, space="PSUM") as rotps:
                k_ps = kvps.tile([128, SK], F32, tag="k")
                v_ps = kvps.tile([128, SK], F32, tag="v")
                for ht in range(HT):
                    if ht == 1:
                        nc.sync.dma_start(xt_sb[:, SK:2 * SK],
                                          xt_d[128:256, :])
                        nc.sync.dma_start(comb_sb[:], comb_d)
                        nc.sync.dma_start(ident_sb[:], ident_d)
                    elif ht >= 2:
                        nc.sync.dma_start(
                            xt_sb[:, SK * ht:SK * (ht + 1)],
                            xt_d[128 * ht:128 * (ht + 1), :])
                    xsl = xt_sb[:, SK * ht:SK * (ht + 1)]
                    for sc in range(3):
                        sl = slice(512 * sc, 512 * (sc + 1))
                        nc.tensor.matmul(k_ps[:, sl],
                                         wkt_sb[:, 128 * ht:128 * (ht + 1)],
                                         xsl[:, sl],
                                         start=(ht == 0), stop=(ht == HT - 1))
                        nc.tensor.matmul(v_ps[:, sl],
                                         wvt_sb[:, 128 * ht:128 * (ht + 1)],
                                         xsl[:, sl],
                                         start=(ht == 0), stop=(ht == HT - 1))
                nc.sync.dma_start(csa_sb[:], csa_d)
                nc.sync.dma_start(csb_sb[:], csb_d)
                nc.sync.dma_start(onesm_sb[:], onesm_d)
                nc.sync.dma_start(bnd_sb[:], bnd_d)
                nc.sync.dma_start(wqt_sb[:], wqt_d)
                nc.sync.dma_start(wot_sb[:], wot_d)
                # rope(k): DVE muls + PE combine (reuses the k psum banks)
                for sc in range(3):
                    sl = slice(512 * sc, 512 * (sc + 1))
                    ma = m1_sb if sc % 2 == 0 else m1b_sb
                    mb = m2_sb if sc % 2 == 0 else m2b_sb
                    nc.vector.tensor_mul(ma[:], k_ps[:, sl], csa_sb[:, sl])
                    nc.vector.tensor_mul(mb[:], k_ps[:, sl], csb_sb[:, sl])
                    nc.tensor.matmul(k_ps[:, sl], comb_sb[:, 0:128], ma[:],
                                     start=True, stop=False)
                    nc.tensor.matmul(k_ps[:, sl], comb_sb[:, 128:256], mb[:],
                                     start=False, stop=True)
                    nc.scalar.copy(kt_rot[:, sl], k_ps[:, sl])
                # v: psum -> sbuf (d-major), transpose to s-major
                for sc in range(3):
                    sl = slice(512 * sc, 512 * (sc + 1))
                    nc.scalar.copy(vt_sb[:, sl], v_ps[:, sl])
                for tj in range(TJ):
                    sl = slice(128 * tj, 128 * (tj + 1))
                    t_ps = rotps.tile([128, 128], BF, tag="tr")
                    nc.tensor.transpose(t_ps[:], vt_sb[:, sl], ident_sb[:])
                    if tj % 2 == 0:
                        nc.vector.tensor_copy(vs_sb[:, sl], t_ps[:])
                    else:
                        nc.scalar.copy(vs_sb[:, sl], t_ps[:])

            # ================ phase 2: q projection + rope =================
            with tc.tile_pool(name="qps", bufs=1, space="PSUM") as qps:
                q_ps = [qps.tile([128, 512], F32, tag=f"q{i}", name=f"q_ps{i}")
                        for i in range(8)]
                for ht in range(HT):
                    for ot in range(NHC):
                        for sc in range(2):
                            nc.tensor.matmul(
                                q_ps[2 * ot + sc][:],
                                wqt_sb[:, 512 * ht + 128 * ot:
                                       512 * ht + 128 * (ot + 1)],
                                xt_sb[:, SK * ht + 512 + 512 * sc:
                                      SK * ht + 512 + 512 * (sc + 1)],
                                start=(ht == 0), stop=(ht == HT - 1))
                for ot in range(NHC):
                    for sc in range(2):
                        i = 2 * ot + sc
                        cs_sl = slice(512 + 512 * sc, 1024 + 512 * sc)
                        ma = m1_sb if i % 2 == 0 else m1b_sb
                        mb = m2_sb if i % 2 == 0 else m2b_sb
                        nc.vector.tensor_mul(ma[:], q_ps[i][:], csa_sb[:, cs_sl])
                        nc.vector.tensor_mul(mb[:], q_ps[i][:], csb_sb[:, cs_sl])
                        nc.tensor.matmul(q_ps[i][:], comb_sb[:, 0:128], ma[:],
                                         start=True, stop=False)
                        nc.tensor.matmul(q_ps[i][:], comb_sb[:, 128:256], mb[:],
                                         start=False, stop=True)
                        nc.scalar.copy(
                            qt_rot[:, SQ * ot + 512 * sc:SQ * ot + 512 * (sc + 1)],
                            q_ps[i][:])

            # ========== phase 3: attention + output projection ==========
            # psum budget: S 3x2 banks + den 1 + o 1 = 8. out-proj f tiles
            # share the S tag/slots.
            with tc.tile_pool(name="att", bufs=1, space="PSUM") as att, \
                 tc.tile_pool(name="pbl", bufs=2) as pbl, \
                 tc.tile_pool(name="ost", bufs=4) as ostp:
                for c in range(2):
                    a = 4 * c
                    pbs = []
                    for h in range(NHC):
                        pb = pbl.tile([128, 2560], BF, tag="pb",
                                      name=f"pb_c{c}h{h}")
                        pbs.append(pb)
                        offs = _pb_off(c)
                        # scores + exp per psum tile
                        for ti, (tw, pieces) in enumerate(_tiles(c)):
                            base = (0, 1024, 2048)[ti]
                            t_ps = att.tile([128, 1024], F32, tag="S", bufs=3,
                                            name=f"s_c{c}h{h}t{ti}")
                            for tj, off in pieces:
                                qlo, n = _piece(tj, c)
                                nc.tensor.matmul(
                                    t_ps[:, off:off + n],
                                    kt_rot[:, 128 * tj:128 * (tj + 1)],
                                    qt_rot[:, SQ * h + qlo:SQ * h + qlo + n],
                                    start=True, stop=True)
                            nc.scalar.activation(
                                pb[:, base:base + tw], t_ps[:, 0:tw], EXP)
                        # boundary masks on exp'd scores
                        # last-128 masks (keep y<=x): pieces a+0..a+3
                        for off in (384, 768, 896, 2048 + 128):
                            nc.vector.tensor_mul(
                                pb[:, off:off + 128], pb[:, off:off + 128],
                                bnd_sb[:, 128:256])
                        # first-128 masks (keep y>=x): pieces a+4..a+7
                        for off in (1024, 1024 + 512, 1024 + 896, 2048 + 256):
                            nc.gpsimd.tensor_mul(
                                pb[:, off:off + 128], pb[:, off:off + 128],
                                bnd_sb[:, 0:128])
                        # den + pv accumulation for this chunk-head
                        o_ps = att.tile([128, 512], F32, tag="o", bufs=1,
                                        name=f"o_c{c}h{h}")
                        den_ps = att.tile([128, 512], F32, tag="den", bufs=1,
                                          name=f"d_c{c}h{h}")
                        order = [a + 3, a + 2, a + 0, a + 4, a + 5, a + 7,
                                 a + 1, a + 6]
                        for idx, tj in enumerate(order):
                            pbo, qlo, n = offs[tj]
                            pc = slice(qlo - 512 * c, qlo - 512 * c + n)
                            prhs = pb[:, pbo:pbo + n]
                            st, sp = idx == 0, idx == len(order) - 1
                            nc.tensor.matmul(den_ps[:, pc],
                                             onesm_sb[:, 128 * tj:128 * (tj + 1)],
                                             prhs, start=st, stop=sp,
                                             skip_group_check=True)
                            nc.tensor.matmul(o_ps[:, pc],
                                             vs_sb[:, 128 * tj:128 * (tj + 1)],
                                             prhs, start=st, stop=sp,
                                             skip_group_check=True)
                        nc.vector.reciprocal_approx_fast(recip_sb[:], den_ps[:])
                        nc.vector.tensor_mul(
                            attnT[:, SQ * h + 512 * c:SQ * h + 512 * (c + 1)],
                            o_ps[:], recip_sb[:])
                    # out projection for this query half (st = 4c..4c+3)
                    for oc in range(4):
                        for sp in range(2):
                            f_ps = att.tile([128, 1024], F32, tag="S", bufs=3,
                                            name=f"f_c{c}o{oc}p{sp}")
                            for m in range(NHC):
                                for j in range(2):
                                    st = 4 * c + 2 * sp + j
                                    nc.tensor.matmul(
                                        f_ps[:, 512 * j:512 * (j + 1)],
                                        attnT[:, SQ * m + 128 * st:
                                              SQ * m + 128 * (st + 1)],
                                        wot_sb[:, 2048 * m + 512 * oc:
                                               2048 * m + 512 * (oc + 1)],
                                        start=(m == 0), stop=(m == NHC - 1))
                            for j in range(2):
                                st = 4 * c + 2 * sp + j
                                stage = ostp.tile([128, 512], BF, tag="st")
                                if j % 2 == 0:
                                    nc.scalar.copy(
                                        stage[:], f_ps[:, 512 * j:512 * (j + 1)])
                                else:
                                    nc.vector.tensor_copy(
                                        stage[:], f_ps[:, 512 * j:512 * (j + 1)])
                                nc.sync.dma_start(
                                    out_d[128 * st:128 * (st + 1),
                                          512 * oc:512 * (oc + 1)],
                                    stage[:])

    nc.compile()
    return nc


def host_inputs(x, wq, wk, wv, wo, freqs_cos, freqs_sin):
    """Build the 8 per-core input dicts (bf16, SBUF-ready layouts)."""
    xT = np.ascontiguousarray(np.asarray(x, dtype=np.float32)[0].T)  # [hid, s]
    wq = np.asarray(wq, dtype=np.float32)
    wk = np.asarray(wk, dtype=np.float32)
    wv = np.asarray(wv, dtype=np.float32)
    wo = np.asarray(wo, dtype=np.float32)
    cosT = np.asarray(freqs_cos, dtype=np.float32).T  # [64, S]
    sinT = np.asarray(freqs_sin, dtype=np.float32).T

    comb = np.zeros((128, 256), dtype=np.float32)
    for p in range(64):
        comb[p, p] = 1.0        # C1: out[p] = m1[p] - m1[p+64]
        comb[64 + p, p] = -1.0
        comb[p, 128 + 64 + p] = 1.0   # C2: out[64+p] = m2[p] + m2[p+64]
        comb[64 + p, 128 + 64 + p] = 1.0
    ident = np.eye(128, dtype=np.float32)
    y = np.arange(128)[None, :]
    xg = np.arange(128)[:, None]
    bnd = np.concatenate([(y >= xg).astype(np.float32),
                          (y <= xg).astype(np.float32)], axis=1)  # [128, 256]

    def bf(a):
        return np.ascontiguousarray(a.astype(BF_NP))

    in_maps = []
    for core in range(NCORES):
        g, r = core // 2, core % 2
        lo = 1024 * r - 512
        xt = np.zeros((HID, SK), dtype=np.float32)
        if r == 0:
            xt[:, 512:] = xT[:, 0:1024]
        else:
            xt[:, :] = xT[:, 512:2048]
        pos = np.clip(np.arange(lo, lo + SK), 0, S - 1)
        csa = np.concatenate([cosT[:, pos], sinT[:, pos]], axis=0)
        csb = np.concatenate([sinT[:, pos], cosT[:, pos]], axis=0)
        onesm = np.zeros((128, SK), dtype=np.float32)
        for tj in range(TJ):
            real = np.ones(128, dtype=np.float32) if r == 1 else \
                (128 * tj + np.arange(128) >= 512).astype(np.float32)
            onesm[:, 128 * tj:128 * (tj + 1)] = real[:, None]
        wqT = wq[512 * g:512 * (g + 1), :].T          # [2048, 512]
        wkT = wk[128 * g:128 * (g + 1), :].T / np.sqrt(HD)  # [2048, 128]
        wvT = wv[128 * g:128 * (g + 1), :].T
        woT = wo[:, 512 * g:512 * (g + 1)].T          # [512, 2048]
        wqt_f = np.concatenate(
            [wqT[128 * t:128 * (t + 1), :] for t in range(HT)], axis=1)
        wkt_f = np.concatenate(
            [wkT[128 * t:128 * (t + 1), :] for t in range(HT)], axis=1)
        wvt_f = np.concatenate(
            [wvT[128 * t:128 * (t + 1), :] for t in range(HT)], axis=1)
        wot_f = np.concatenate(
            [woT[128 * m:128 * (m + 1), 512 * oc:512 * (oc + 1)]
             for m in range(NHC) for oc in range(4)], axis=1)
        in_maps.append({
            "xt": bf(xt),
            "wqt": bf(wqt_f),
            "wkt": bf(wkt_f),
            "wvt": bf(wvt_f),
            "wot": bf(wot_f),
            "csa": np.ascontiguousarray(csa),
            "csb": np.ascontiguousarray(csb),
            "comb": bf(comb),
            "ident": bf(ident),
            "onesm": bf(onesm),
            "bnd": bf(bnd),
        })
    return in_maps


def reduce_outputs(results):
    out = np.zeros((S, HID), dtype=np.float32)
    for core, res in enumerate(results):
        r = core % 2
        out[1024 * r:1024 * (r + 1), :] += np.asarray(res["out"],
                                                      dtype=np.float32)
    return out[None]


_NC = None
_IN_MAPS = None


def _numpy_fallback(x, wq, wk, wv, wo, attention_mask, freqs_cos, freqs_sin):
    """Exact (slow) path for non-causal attention_mask inputs."""
    xs = np.asarray(x, np.float32)[0]
    cos = np.asarray(freqs_cos, np.float32)
    sin = np.asarray(freqs_sin, np.float32)

    def rope(t):
        x1, x2 = t[..., :64], t[..., 64:]
        c, s = cos[:, None, :], sin[:, None, :]
        return np.concatenate([x1 * c - x2 * s, x1 * s + x2 * c], axis=-1)

    q = rope((xs @ np.asarray(wq, np.float32).T).reshape(S, NH, HD))
    k = rope((xs @ np.asarray(wk, np.float32).T).reshape(S, NKV, HD))
    v = (xs @ np.asarray(wv, np.float32).T).reshape(S, NKV, HD)
    k = np.repeat(k, NH // NKV, axis=1)
    v = np.repeat(v, NH // NKV, axis=1)
    i = np.arange(S)[:, None]
    j = np.arange(S)[None, :]
    wmask = (i - j) > WINDOW
    out = np.zeros((S, NH, HD), np.float32)
    am = np.asarray(attention_mask, np.float32)[0, 0]
    for h in range(NH):
        sc = (q[:, h] @ k[:, h].T) / np.sqrt(HD) + am
        sc = np.where(wmask, -np.inf, sc)
        sc -= sc.max(axis=1, keepdims=True)
        p = np.exp(sc)
        p /= p.sum(axis=1, keepdims=True)
        out[:, h] = p @ v[:, h]
    return (out.reshape(S, NH * HD) @ np.asarray(wo, np.float32).T)[None]


def _is_standard_causal(attention_mask):
    am = np.asarray(attention_mask)
    if am.shape != (1, 1, S, S):
        return False
    i = np.arange(S)[:, None]
    j = np.arange(S)[None, :]
    expect = np.where(j > i, np.float32(-1e9), np.float32(0.0))
    return np.array_equal(am[0, 0], expect)


def kernel(x, wq, wk, wv, wo, attention_mask, freqs_cos, freqs_sin,
           **extra):
    global _NC, _IN_MAPS
    if not _is_standard_causal(attention_mask):
        return _numpy_fallback(x, wq, wk, wv, wo, attention_mask,
                               freqs_cos, freqs_sin)
    in_maps = host_inputs(x, wq, wk, wv, wo, freqs_cos, freqs_sin)
    _IN_MAPS = in_maps
    if _NC is None:
        _NC = build_nc()
    res = run_bass_kernel_spmd(_NC, in_maps, core_ids=list(range(NCORES)))
    return reduce_outputs(res.results)


if __name__ == "__main__":
    nc = build_nc()
    print("kernel built OK")


# revision 6
# speedup vs baseline: 1.2468x; 1.1388x over previous
"""Trainium2 Bass kernel for sliding-window GQA attention block (bf16).

Problem: B=1, S=2048, HID=2048, NH=16 q-heads, NKV=4 kv-heads, HD=128,
WINDOW=512, causal; rotary embedding on q/k; projections wq/wk/wv/wo.

Sharding (8 cores): tensor-parallel over the 4 KV-head groups (4 q-heads
per group) x sequence-parallel over 2 halves of 1024 queries. Each core
computes its group's q/k/v projections for its sequence span (+512-key
halo), banded sliding-window attention, and a partial output projection.
Host sums the 4 group-partials per half.

v2 (bf16 rewrite of the fp32r baseline):
- All matmul operands bf16 (fp32 PSUM accumulation). Halves HBM traffic
  and removes the fp32r N<256 4x matmul penalty.
- x is DMA'd once (bf16, host-transposed) and stays resident in SBUF;
  both the k/v and q projections stream it from there.
- Weights are pre-laid out on the host in the exact SBUF layout so every
  DMA is a single contiguous transfer (the fp32r baseline lost ~19us to
  strided weight gathers before the first matmul).
- Scores for each (chunk, head) are packed into three PSUM tiles
  (1024/1024/512 wide, pieces bank-aligned) so exp() runs as 3 large
  ACT calls instead of 12 small ones.
- Chunk-outer / head-inner attention ordering: the output projection for
  query-half 0 overlaps attention for query-half 1 in PSUM.
- Output partials are bf16 (host accumulates in fp32).
"""
import sys
import os

sys.path.insert(0, "/opt/trn_rl_repo")

import numpy as np
import ml_dtypes

import concourse.bass as bass
import concourse.mybir as mybir
from concourse import bacc
import concourse.tile as tile
from concourse.bass_utils import run_bass_kernel_spmd

BF = mybir.dt.bfloat16
F32 = mybir.dt.float32

S, HID, NH, NKV, HD, WINDOW = 2048, 2048, 16, 4, 128, 512
NCORES = 8
SQ = 1024          # queries per core
SK = 1536          # keys per core (incl. 512 halo)
HT = HID // 128    # 16 hid tiles
NHC = NH // NKV    # 4 q-heads per core
TJ = SK // 128     # 12 key tiles
EXP = mybir.ActivationFunctionType.Exp
BF_NP = ml_dtypes.bfloat16


def _win(tj):
    """Query window [w0, w1) of key tile tj in core-local coordinates."""
    return max(0, 128 * tj - 512), min(SQ, 128 * tj + 128)


def _piece(tj, c):
    """Clip key-tile tj's query window to chunk c. -> (qlo, n) or None."""
    w0, w1 = _win(tj)
    lo = max(0, 512 * c - w0)
    hi = min(w1 - w0, 512 * c + 512 - w0)
    if hi <= lo:
        return None
    return w0 + lo, hi - lo


# Scores/pblock layout per chunk c (a=4c): three bank-aligned psum tiles.
# T0: [a+3:512 @0 | a+2:384 @512, a+0:128 @896]
# T1: [a+4:512 @0 | a+5:384 @512, a+7:128 @896]
# T2: [a+1:256 @0 | a+6:256 @256]
# pb offsets: T0 -> +0, T1 -> +1024, T2 -> +2048 (2560 per chunk-head).
def _tiles(c):
    a = 4 * c
    return [
        (1024, [(a + 3, 0), (a + 2, 512), (a + 0, 896)]),
        (1024, [(a + 4, 0), (a + 5, 512), (a + 7, 896)]),
        (512, [(a + 1, 0), (a + 6, 256)]),
    ]


# (pb_offset, n) inside the 2560-wide chunk block, by tj (for den/pv).
def _pb_off(c):
    out = {}
    for ti, (_, pieces) in enumerate(_tiles(c)):
        base = (0, 1024, 2048)[ti]
        for tj, off in pieces:
            qlo, n = _piece(tj, c)
            out[tj] = (base + off, qlo, n)
    return out


def build_nc():
    nc = bacc.Bacc("TRN2", target_bir_lowering=False, debug=False)

    xt_d = nc.dram_tensor("xt", [HID, SK], BF, kind="ExternalInput").ap()
    wqt_d = nc.dram_tensor("wqt", [128, HT * 512], BF, kind="ExternalInput").ap()
    wkt_d = nc.dram_tensor("wkt", [128, HT * 128], BF, kind="ExternalInput").ap()
    wvt_d = nc.dram_tensor("wvt", [128, HT * 128], BF, kind="ExternalInput").ap()
    wot_d = nc.dram_tensor("wot", [128, 16 * 512], BF, kind="ExternalInput").ap()
    csa_d = nc.dram_tensor("csa", [128, SK], F32, kind="ExternalInput").ap()
    csb_d = nc.dram_tensor("csb", [128, SK], F32, kind="ExternalInput").ap()
    comb_d = nc.dram_tensor("comb", [128, 256], BF, kind="ExternalInput").ap()
    ident_d = nc.dram_tensor("ident", [128, 128], BF, kind="ExternalInput").ap()
    onesm_d = nc.dram_tensor("onesm", [128, SK], BF, kind="ExternalInput").ap()
    bnd_d = nc.dram_tensor("bnd", [128, 256], BF, kind="ExternalInput").ap()
    out_d = nc.dram_tensor("out", [SQ, HID], BF, kind="ExternalOutput").ap()

    with tile.TileContext(nc) as tc:
        with tc.tile_pool(name="persist", bufs=1) as pp:
            wkt_sb = pp.tile([128, HT * 128], BF)
            wvt_sb = pp.tile([128, HT * 128], BF)
            wqt_sb = pp.tile([128, HT * 512], BF)
            wot_sb = pp.tile([128, 16 * 512], BF)
            xt_sb = pp.tile([128, HT * SK], BF)     # 48KB/part, resident x
            csa_sb = pp.tile([128, SK], F32)
            csb_sb = pp.tile([128, SK], F32)
            comb_sb = pp.tile([128, 256], BF)
            ident_sb = pp.tile([128, 128], BF)
            onesm_sb = pp.tile([128, SK], BF)
            bnd_sb = pp.tile([128, 256], BF)
            kt_rot = pp.tile([128, SK], BF)
            vs_sb = pp.tile([128, SK], BF)
            vt_sb = pp.tile([128, SK], BF)
            qt_rot = pp.tile([128, NHC * SQ], BF)
            attnT = pp.tile([128, NHC * SQ], BF)
            m1_sb = pp.tile([128, 512], BF)
            m2_sb = pp.tile([128, 512], BF)
            m1b_sb = pp.tile([128, 512], BF)
            m2b_sb = pp.tile([128, 512], BF)
            recip_sb = pp.tile([128, 512], F32)

            # ---- priming DMAs (order = consumption order; first chunks
            # small so the first matmul can issue ASAP) ----
            nc.sync.dma_start(wkt_sb[:, 0:1024], wkt_d[:, 0:1024])
            nc.sync.dma_start(xt_sb[:, 0:512], xt_d[0:128, 0:512])
            nc.sync.dma_start(wvt_sb[:, 0:1024], wvt_d[:, 0:1024])
            nc.sync.dma_start(xt_sb[:, 512:SK], xt_d[0:128, 512:SK])
            nc.sync.dma_start(wkt_sb[:, 1024:2048], wkt_d[:, 1024:2048])
            nc.sync.dma_start(wvt_sb[:, 1024:2048], wvt_d[:, 1024:2048])

            # ================= phase 1: k/v projections =================
            with tc.tile_pool(name="kvps", bufs=1, space="PSUM") as kvps, \
                 tc.tile_pool(name="rotps", bufs=2, space="PSUM") as rotps:
                k_ps = kvps.tile([128, SK], F32, tag="k")
                v_ps = kvps.tile([128, SK], F32, tag="v")
                for ht in range(HT):
                    if ht == 1:
                        nc.sync.dma_start(xt_sb[:, SK:2 * SK],
                                          xt_d[128:256, :])
                        nc.sync.dma_start(comb_sb[:], comb_d)
                        nc.sync.dma_start(ident_sb[:], ident_d)
                    elif ht >= 2:
                        nc.sync.dma_start(
                            xt_sb[:, SK * ht:SK * (ht + 1)],
                            xt_d[128 * ht:128 * (ht + 1), :])
                    xsl = xt_sb[:, SK * ht:SK * (ht + 1)]
                    for sc in range(3):
                        sl = slice(512 * sc, 512 * (sc + 1))
                        nc.tensor.matmul(k_ps[:, sl],
                                         wkt_sb[:, 128 * ht:128 * (ht + 1)],
                                         xsl[:, sl],
                                         start=(ht == 0), stop=(ht == HT - 1))
                        nc.tensor.matmul(v_ps[:, sl],
                                         wvt_sb[:, 128 * ht:128 * (ht + 1)],
                                         xsl[:, sl],
                                         start=(ht == 0), stop=(ht == HT - 1))
                nc.sync.dma_start(csa_sb[:], csa_d)
                nc.sync.dma_start(csb_sb[:], csb_d)
                nc.sync.dma_start(onesm_sb[:], onesm_d)
                nc.sync.dma_start(bnd_sb[:], bnd_d)
                nc.sync.dma_start(wqt_sb[:], wqt_d)
                nc.sync.dma_start(wot_sb[:], wot_d)
                # v evac first (frees v banks; ACT work while DVE starts rope)
                for sc in range(3):
                    sl = slice(512 * sc, 512 * (sc + 1))
                    nc.scalar.copy(vt_sb[:, sl], v_ps[:, sl])
                # rope(k) muls on DVE; PE fills the wait with v transposes
                for sc in range(3):
                    sl = slice(512 * sc, 512 * (sc + 1))
                    ma = m1_sb if sc % 2 == 0 else m1b_sb
                    mb = m2_sb if sc % 2 == 0 else m2b_sb
                    nc.vector.tensor_mul(ma[:], k_ps[:, sl], csa_sb[:, sl])
                    nc.vector.tensor_mul(mb[:], k_ps[:, sl], csb_sb[:, sl])
                    for tj in range(4 * sc, 4 * sc + 4):
                        tsl = slice(128 * tj, 128 * (tj + 1))
                        t_ps = rotps.tile([128, 128], BF, tag="tr")
                        nc.tensor.transpose(t_ps[:], vt_sb[:, tsl], ident_sb[:])
                        if tj % 2 == 0:
                            nc.vector.tensor_copy(vs_sb[:, tsl], t_ps[:])
                        else:
                            nc.scalar.copy(vs_sb[:, tsl], t_ps[:])
                    nc.tensor.matmul(k_ps[:, sl], comb_sb[:, 0:128], ma[:],
                                     start=True, stop=False)
                    nc.tensor.matmul(k_ps[:, sl], comb_sb[:, 128:256], mb[:],
                                     start=False, stop=True)
                    nc.scalar.copy(kt_rot[:, sl], k_ps[:, sl])

            # ===== phase 2: q projection + rope, 4 passes of 2 banks =====
            # Each pass handles one head (2x512 queries) so PE can start as
            # soon as 2 psum banks free up, overlapping the k-rope tail.
            with tc.tile_pool(name="qps", bufs=1, space="PSUM") as qps:
                for ot in range(NHC):
                    qt_ps = qps.tile([128, 1024], F32, tag="q", bufs=2,
                                     name=f"q_ps{ot}")
                    for ht in range(HT):
                        for sc in range(2):
                            nc.tensor.matmul(
                                qt_ps[:, 512 * sc:512 * (sc + 1)],
                                wqt_sb[:, 512 * ht + 128 * ot:
                                       512 * ht + 128 * (ot + 1)],
                                xt_sb[:, SK * ht + 512 + 512 * sc:
                                      SK * ht + 512 + 512 * (sc + 1)],
                                start=(ht == 0), stop=(ht == HT - 1))
                    for sc in range(2):
                        sl = slice(512 * sc, 512 * (sc + 1))
                        cs_sl = slice(512 + 512 * sc, 1024 + 512 * sc)
                        ma = m1_sb if sc % 2 == 0 else m1b_sb
                        mb = m2_sb if sc % 2 == 0 else m2b_sb
                        nc.vector.tensor_mul(ma[:], qt_ps[:, sl], csa_sb[:, cs_sl])
                        nc.vector.tensor_mul(mb[:], qt_ps[:, sl], csb_sb[:, cs_sl])
                        nc.tensor.matmul(qt_ps[:, sl], comb_sb[:, 0:128], ma[:],
                                         start=True, stop=False)
                        nc.tensor.matmul(qt_ps[:, sl], comb_sb[:, 128:256], mb[:],
                                         start=False, stop=True)
                        nc.scalar.copy(
                            qt_rot[:, SQ * ot + 512 * sc:SQ * ot + 512 * (sc + 1)],
                            qt_ps[:, sl])

            # ========== phase 3: attention + output projection ==========
            # psum budget: S 3x2 banks + den 1 + o 1 = 8. out-proj f tiles
            # share the S tag/slots.
            with tc.tile_pool(name="att", bufs=1, space="PSUM") as att, \
                 tc.tile_pool(name="pbl", bufs=2) as pbl, \
                 tc.tile_pool(name="ost", bufs=4) as ostp:
                for c in range(2):
                    a = 4 * c
                    pbs = []
                    for h in range(NHC):
                        pb = pbl.tile([128, 2560], BF, tag="pb",
                                      name=f"pb_c{c}h{h}")
                        pbs.append(pb)
                        offs = _pb_off(c)
                        # scores + exp per psum tile
                        for ti, (tw, pieces) in enumerate(_tiles(c)):
                            base = (0, 1024, 2048)[ti]
                            t_ps = att.tile([128, 1024], F32, tag="S", bufs=2,
                                            name=f"s_c{c}h{h}t{ti}")
                            for tj, off in pieces:
                                qlo, n = _piece(tj, c)
                                nc.tensor.matmul(
                                    t_ps[:, off:off + n],
                                    kt_rot[:, 128 * tj:128 * (tj + 1)],
                                    qt_rot[:, SQ * h + qlo:SQ * h + qlo + n],
                                    start=True, stop=True)
                            nc.scalar.activation(
                                pb[:, base:base + tw], t_ps[:, 0:tw], EXP)
                        # boundary masks on exp'd scores
                        # last-128 masks (keep y<=x): pieces a+0..a+3
                        for off in (384, 768, 896, 2048 + 128):
                            nc.vector.tensor_mul(
                                pb[:, off:off + 128], pb[:, off:off + 128],
                                bnd_sb[:, 128:256])
                        # first-128 masks (keep y>=x): pieces a+4..a+7
                        for off in (1024, 1024 + 512, 1024 + 896, 2048 + 256):
                            nc.gpsimd.tensor_mul(
                                pb[:, off:off + 128], pb[:, off:off + 128],
                                bnd_sb[:, 0:128])
                        # den + pv accumulation for this chunk-head
                        o_ps = att.tile([128, 512], F32, tag="o", bufs=2,
                                        name=f"o_c{c}h{h}")
                        den_ps = att.tile([128, 512], F32, tag="den", bufs=2,
                                          name=f"d_c{c}h{h}")
                        order = [a + 3, a + 2, a + 0, a + 4, a + 5, a + 7,
                                 a + 1, a + 6]
                        for idx, tj in enumerate(order):
                            pbo, qlo, n = offs[tj]
                            pc = slice(qlo - 512 * c, qlo - 512 * c + n)
                            prhs = pb[:, pbo:pbo + n]
                            st, sp = idx == 0, idx == len(order) - 1
                            nc.tensor.matmul(den_ps[:, pc],
                                             onesm_sb[:, 128 * tj:128 * (tj + 1)],
                                             prhs, start=st, stop=sp,
                                             skip_group_check=True)
                            nc.tensor.matmul(o_ps[:, pc],
                                             vs_sb[:, 128 * tj:128 * (tj + 1)],
                                             prhs, start=st, stop=sp,
                                             skip_group_check=True)
                        nc.vector.reciprocal_approx_fast(recip_sb[:], den_ps[:])
                        nc.vector.tensor_mul(
                            attnT[:, SQ * h + 512 * c:SQ * h + 512 * (c + 1)],
                            o_ps[:], recip_sb[:])
                    # out projection for this query half (st = 4c..4c+3)
                    for oc in range(4):
                        for sp in range(2):
                            f_ps = att.tile([128, 1024], F32, tag="S", bufs=2,
                                            name=f"f_c{c}o{oc}p{sp}")
                            for m in range(NHC):
                                for j in range(2):
                                    st = 4 * c + 2 * sp + j
                                    nc.tensor.matmul(
                                        f_ps[:, 512 * j:512 * (j + 1)],
                                        attnT[:, SQ * m + 128 * st:
                                              SQ * m + 128 * (st + 1)],
                                        wot_sb[:, 2048 * m + 512 * oc:
                                               2048 * m + 512 * (oc + 1)],
                                        start=(m == 0), stop=(m == NHC - 1))
                            for j in range(2):
                                st = 4 * c + 2 * sp + j
                                stage = ostp.tile([128, 512], BF, tag="st")
                                if j % 2 == 0:
                                    nc.scalar.copy(
                                        stage[:], f_ps[:, 512 * j:512 * (j + 1)])
                                else:
                                    nc.vector.tensor_copy(
                                        stage[:], f_ps[:, 512 * j:512 * (j + 1)])
                                nc.sync.dma_start(
                                    out_d[128 * st:128 * (st + 1),
                                          512 * oc:512 * (oc + 1)],
                                    stage[:])

    nc.compile()
    return nc


def host_inputs(x, wq, wk, wv, wo, freqs_cos, freqs_sin):
    """Build the 8 per-core input dicts (bf16, SBUF-ready layouts)."""
    xT = np.ascontiguousarray(np.asarray(x, dtype=np.float32)[0].T)  # [hid, s]
    wq = np.asarray(wq, dtype=np.float32)
    wk = np.asarray(wk, dtype=np.float32)
    wv = np.asarray(wv, dtype=np.float32)
    wo = np.asarray(wo, dtype=np.float32)
    cosT = np.asarray(freqs_cos, dtype=np.float32).T  # [64, S]
    sinT = np.asarray(freqs_sin, dtype=np.float32).T

    comb = np.zeros((128, 256), dtype=np.float32)
    for p in range(64):
        comb[p, p] = 1.0        # C1: out[p] = m1[p] - m1[p+64]
        comb[64 + p, p] = -1.0
        comb[p, 128 + 64 + p] = 1.0   # C2: out[64+p] = m2[p] + m2[p+64]
        comb[64 + p, 128 + 64 + p] = 1.0
    ident = np.eye(128, dtype=np.float32)
    y = np.arange(128)[None, :]
    xg = np.arange(128)[:, None]
    bnd = np.concatenate([(y >= xg).astype(np.float32),
                          (y <= xg).astype(np.float32)], axis=1)  # [128, 256]

    def bf(a):
        return np.ascontiguousarray(a.astype(BF_NP))

    in_maps = []
    for core in range(NCORES):
        g, r = core // 2, core % 2
        lo = 1024 * r - 512
        xt = np.zeros((HID, SK), dtype=np.float32)
        if r == 0:
            xt[:, 512:] = xT[:, 0:1024]
        else:
            xt[:, :] = xT[:, 512:2048]
        pos = np.clip(np.arange(lo, lo + SK), 0, S - 1)
        csa = np.concatenate([cosT[:, pos], sinT[:, pos]], axis=0)
        csb = np.concatenate([sinT[:, pos], cosT[:, pos]], axis=0)
        onesm = np.zeros((128, SK), dtype=np.float32)
        for tj in range(TJ):
            real = np.ones(128, dtype=np.float32) if r == 1 else \
                (128 * tj + np.arange(128) >= 512).astype(np.float32)
            onesm[:, 128 * tj:128 * (tj + 1)] = real[:, None]
        wqT = wq[512 * g:512 * (g + 1), :].T          # [2048, 512]
        wkT = wk[128 * g:128 * (g + 1), :].T / np.sqrt(HD)  # [2048, 128]
        wvT = wv[128 * g:128 * (g + 1), :].T
        woT = wo[:, 512 * g:512 * (g + 1)].T          # [512, 2048]
        wqt_f = np.concatenate(
            [wqT[128 * t:128 * (t + 1), :] for t in range(HT)], axis=1)
        wkt_f = np.concatenate(
            [wkT[128 * t:128 * (t + 1), :] for t in range(HT)], axis=1)
        wvt_f = np.concatenate(
            [wvT[128 * t:128 * (t + 1), :] for t in range(HT)], axis=1)
        wot_f = np.concatenate(
            [woT[128 * m:128 * (m + 1), 512 * oc:512 * (oc + 1)]
             for m in range(NHC) for oc in range(4)], axis=1)
        in_maps.append({
            "xt": bf(xt),
            "wqt": bf(wqt_f),
            "wkt": bf(wkt_f),
            "wvt": bf(wvt_f),
            "wot": bf(wot_f),
            "csa": np.ascontiguousarray(csa),
            "csb": np.ascontiguousarray(csb),
            "comb": bf(comb),
            "ident": bf(ident),
            "onesm": bf(onesm),
            "bnd": bf(bnd),
        })
    return in_maps


def reduce_outputs(results):
    out = np.zeros((S, HID), dtype=np.float32)
    for core, res in enumerate(results):
        r = core % 2
        out[1024 * r:1024 * (r + 1), :] += np.asarray(res["out"],
                                                      dtype=np.float32)
    return out[None]


_NC = None
_IN_MAPS = None


def _numpy_fallback(x, wq, wk, wv, wo, attention_mask, freqs_cos, freqs_sin):
    """Exact (slow) path for non-causal attention_mask inputs."""
    xs = np.asarray(x, np.float32)[0]
    cos = np.asarray(freqs_cos, np.float32)
    sin = np.asarray(freqs_sin, np.float32)

    def rope(t):
        x1, x2 = t[..., :64], t[..., 64:]
        c, s = cos[:, None, :], sin[:, None, :]
        return np.concatenate([x1 * c - x2 * s, x1 * s + x2 * c], axis=-1)

    q = rope((xs @ np.asarray(wq, np.float32).T).reshape(S, NH, HD))
    k = rope((xs @ np.asarray(wk, np.float32).T).reshape(S, NKV, HD))
    v = (xs @ np.asarray(wv, np.float32).T).reshape(S, NKV, HD)
    k = np.repeat(k, NH // NKV, axis=1)
    v = np.repeat(v, NH // NKV, axis=1)
    i = np.arange(S)[:, None]
    j = np.arange(S)[None, :]
    wmask = (i - j) > WINDOW
    out = np.zeros((S, NH, HD), np.float32)
    am = np.asarray(attention_mask, np.float32)[0, 0]
    for h in range(NH):
        sc = (q[:, h] @ k[:, h].T) / np.sqrt(HD) + am
        sc = np.where(wmask, -np.inf, sc)
        sc -= sc.max(axis=1, keepdims=True)
        p = np.exp(sc)
        p /= p.sum(axis=1, keepdims=True)
        out[:, h] = p @ v[:, h]
    return (out.reshape(S, NH * HD) @ np.asarray(wo, np.float32).T)[None]


def _is_standard_causal(attention_mask):
    am = np.asarray(attention_mask)
    if am.shape != (1, 1, S, S):
        return False
    i = np.arange(S)[:, None]
    j = np.arange(S)[None, :]
    expect = np.where(j > i, np.float32(-1e9), np.float32(0.0))
    return np.array_equal(am[0, 0], expect)


def kernel(x, wq, wk, wv, wo, attention_mask, freqs_cos, freqs_sin,
           **extra):
    global _NC, _IN_MAPS
    if not _is_standard_causal(attention_mask):
        return _numpy_fallback(x, wq, wk, wv, wo, attention_mask,
                               freqs_cos, freqs_sin)
    in_maps = host_inputs(x, wq, wk, wv, wo, freqs_cos, freqs_sin)
    _IN_MAPS = in_maps
    if _NC is None:
        _NC = build_nc()
    res = run_bass_kernel_spmd(_NC, in_maps, core_ids=list(range(NCORES)))
    return reduce_outputs(res.results)


if __name__ == "__main__":
    nc = build_nc()
    print("kernel built OK")
